# revision 25
# baseline (speedup 1.0000x reference)
"""Trainium2 Bass kernel for the FFF (fast feedforward / MoE-routing) module.

Math (per token x of dim 1024, PAR=8 trees of 255 nodes):
  logits = x @ W_in.T + b_in                      # [B, 2040]
  dec    = logits > 0
  acts   = silu(logits)
  dmap   = indicator of the 8 visited nodes per tree
  out    = (acts * dmap) @ W_out.T                # [B, 1024]

Strategy (8 NeuronCores, data-parallel over the 8192 tokens, 1024 each):
  - Decision region (cols 0..1024) in fp32r (TF32-like: PE rounds both
    operands to 10 explicit mantissa bits at ~bf16 speed).  Inputs are
    pre-rounded to 10 bits on the host so the fp32r pass is EXACT; the
    known residual dx@w + x@dw is computed via fp8e4 DoubleRow matmuls
    (2 planes/instr, 2x rate at M=128) into a separate PSUM and merged at
    the bias-add.  Main pass pre-scaled by 2^17 (x*2^8, w*2^9) to match
    the fp8 plane scales; silu uses the activation unit's input scale.
    Net logit error ~5e-6 rms -> ~0 decision flips vs fp32 reference.
  - Leaf region (cols 1024..2048) single-pass bf16 (acts only need ~1e-3).
  - dmap built level-by-level with strided vector ops in node-major
    column layout (col = 8*node + tree).
  - masked acts in fp16, transposed on the PE, GEMM2 in fp16.
  - DMAs emitted in need-order; tiles 0-1 are hand-interleaved with the
    weight-arrival schedule so the PE has work while weights stream.
"""

import numpy as np
import ml_dtypes

DIM = 1024
PAR = 8
DEPTH = 7
N_NODES = 255
WIDTH = PAR * N_NODES          # 2040
NODES_PAD = 2048
N_CORES = 8
TOK_PER_CORE = 1024
TT = 128
NTILES = TOK_PER_CORE // TT    # 8
K_CH = DIM // 128              # 8
C_CH = NODES_PAD // 128        # 16
DEC_COLS = 8 * 127             # 1016
CORR_COLS = 1024               # fp8-corrected region (blocks 0,1)
SC = float(2 ** 17)

_PROGRAM = None


def _build_program():
    import concourse.bacc as bacc
    import concourse.tile as tile
    from concourse import mybir
    from concourse.masks import make_identity
    import concourse.bass as bass

    f32 = mybir.dt.float32
    f32r = mybir.dt.float32r
    bf16 = mybir.dt.bfloat16
    f16 = mybir.dt.float16
    fp8e4 = mybir.dt.float8e4
    Alu = mybir.AluOpType
    Act = mybir.ActivationFunctionType
    DRM = mybir.MatmulPerfMode.DoubleRow

    nc = bacc.Bacc("TRN2", target_bir_lowering=False, debug=False,
                   num_devices=N_CORES)

    xs = nc.dram_tensor("xs", [128, NTILES, K_CH, TT], f32r,
                        kind="ExternalInput")
    x8p = nc.dram_tensor("x8p", [128, NTILES, K_CH, 2, TT], fp8e4,
                         kind="ExternalInput")
    xh = nc.dram_tensor("xh", [128, NTILES, K_CH, TT], bf16,
                        kind="ExternalInput")
    w1r = nc.dram_tensor("w1r", [128, K_CH, CORR_COLS], f32r,
                         kind="ExternalInput")
    w8p = nc.dram_tensor("w8p", [128, K_CH, 2, CORR_COLS], fp8e4,
                         kind="ExternalInput")
    w1h = nc.dram_tensor("w1h", [128, K_CH, 1024], bf16,
                         kind="ExternalInput")
    b1s = nc.dram_tensor("b1s", [NODES_PAD], f32, kind="ExternalInput")
    w2 = nc.dram_tensor("w2", [128, C_CH, DIM], f16, kind="ExternalInput")
    y = nc.dram_tensor("y", [TOK_PER_CORE, DIM], f32, kind="ExternalOutput")

    with tile.TileContext(nc) as tc:
        with (
            tc.tile_pool(name="wts", bufs=1) as wts,
            tc.tile_pool(name="xts", bufs=4) as xts,
            tc.tile_pool(name="lgs", bufs=2) as lgs_pool,
            tc.tile_pool(name="d1p", bufs=4) as d1_pool,
            tc.tile_pool(name="vvp", bufs=2) as vv_pool,
            tc.tile_pool(name="acp", bufs=4) as ac_pool,
            tc.tile_pool(name="mkp", bufs=2) as mk_pool,
            tc.tile_pool(name="out", bufs=2) as out_pool,
            tc.tile_pool(name="pl", bufs=3, space="PSUM") as pl_pool,
            tc.tile_pool(name="pc", bufs=2, space="PSUM") as pc_pool,
            tc.tile_pool(name="pt", bufs=2, space="PSUM") as pt_pool,
            tc.tile_pool(name="py", bufs=1, space="PSUM") as py_pool,
        ):
            # Weight tiles are split per DMA batch: the Tile framework
            # tracks dependencies at tile granularity, so a consumer waits
            # for ALL writes to its tile — separate tiles let the first
            # matmuls start as soon as their own bytes land.
            w1r_b0k0 = wts.tile([128, 1, 512], f32r)      # b0, k 0
            w1r_b0a = wts.tile([128, 3, 512], f32r)       # b0, k 1-3
            w1r_b0b = wts.tile([128, 4, 512], f32r)       # b0, k 4-7
            w1r_b1 = wts.tile([128, K_CH, 512], f32r)
            w8p_a = wts.tile([128, 4, 2, CORR_COLS], fp8e4)   # k 0-3
            w8p_b = wts.tile([128, 4, 2, CORR_COLS], fp8e4)   # k 4-7
            w1h_b2 = wts.tile([128, K_CH, 512], bf16)
            w1h_b3 = wts.tile([128, K_CH, 512], bf16)
            w2_sb = wts.tile([128, C_CH, DIM], f16)
            b1s_sb = wts.tile([128, NODES_PAD], f32)
            ident = wts.tile([128, 128], f16)

            def w1r_at(k, b):
                if b == 1:
                    return w1r_b1[:, k, :]
                if k == 0:
                    return w1r_b0k0[:, 0, :]
                return (w1r_b0a[:, k - 1, :] if k < 4 else w1r_b0b[:, k - 4, :])

            def w8p_at(k, cols):
                t = w8p_a if k < 4 else w8p_b
                return t[:, k % 4, :, cols]

            def w1h_at(k, b):
                return (w1h_b2 if b == 2 else w1h_b3)[:, k, :]

            xt_tiles = {}

            def fetch_xs(j, eng):
                xsj = xts.tile([128, K_CH, TT], f32r, tag="xs")
                eng.dma_start(out=xsj, in_=xs[:, j])
                return xsj

            def fetch_x8(j, eng):
                x8j = xts.tile([128, K_CH, 2, TT], fp8e4, tag="x8")
                eng.dma_start(out=x8j, in_=x8p[:, j])
                return x8j

            def fetch_xh(j, eng):
                xhj = xts.tile([128, K_CH, TT], bf16, tag="xh")
                eng.dma_start(out=xhj, in_=xh[:, j])
                return xhj

            def prefetch_xt(j, eng=None):
                e = eng or nc.gpsimd
                xt_tiles[j] = (fetch_xs(j, e), fetch_x8(j, e), fetch_xh(j, e))

            # bias broadcast + identity off the Sync weight path, early
            b1_bcast = bass.AP(tensor=b1s, offset=0,
                               ap=[[0, 128], [1, NODES_PAD]])
            nc.gpsimd.dma_start(out=b1s_sb, in_=b1_bcast)
            make_identity(nc, ident)

            # per-tile stage-A state
            st = {}

            def a_init(j):
                if j not in xt_tiles:
                    prefetch_xt(j)
                d1 = d1_pool.tile([TT, DEC_COLS], f16, tag="d1")
                vv = vv_pool.tile([TT, WIDTH], f16, tag="vv")
                ac = ac_pool.tile([TT, NODES_PAD], f16, tag="ac")
                st[j] = {"x": xt_tiles.pop(j), "d1": d1, "vv": vv, "ac": ac,
                         "pl": {}, "pc": {}}

            def a_main(j, b):
                """fp32r main pass for block b (512 cols), group closed."""
                s = st[j]
                xsat = s.get("xsat") or (lambda k: s["x"][0][:, k, :])
                pl = pl_pool.tile([TT, 512], f32)
                for k in range(K_CH):
                    nc.tensor.matmul(pl, lhsT=xsat(k),
                                     rhs=w1r_at(k, b),
                                     start=(k == 0), stop=(k == K_CH - 1))
                s["pl"][b] = pl

            def a_corr(j, b):
                """fp8 DR correction for block b into its own PSUM."""
                s = st[j]
                x8j = s["x"][1]
                lo = b * 512
                pc = pc_pool.tile([TT, 512], f32)
                for q in range(2):
                    qs = slice(q * 256, (q + 1) * 256)
                    wq = slice(lo + q * 256, lo + (q + 1) * 256)
                    for k in range(K_CH):
                        nc.tensor.matmul(
                            pc[:, qs], lhsT=x8j[:, k, :, :],
                            rhs=w8p_at(k, wq),
                            start=(k == 0), stop=(k == K_CH - 1),
                            perf_mode=DRM, skip_group_check=True)
                s["pc"][b] = pc

            def a_leaf(j, b):
                """bf16 leaf pass for block b (2 or 3), group closed."""
                s = st[j]
                xhj = s["x"][2]
                pl = pl_pool.tile([TT, 512], f32)
                for k in range(K_CH):
                    nc.tensor.matmul(pl, lhsT=xhj[:, k, :],
                                     rhs=w1h_at(k, b),
                                     start=(k == 0), stop=(k == K_CH - 1))
                s["pl"][b] = pl

            def a_epi(j, b):
                """bias (+corr) add, decisions, silu for block b."""
                s = st[j]
                lo = b * 512
                pl = s["pl"].pop(b)
                lg = lgs_pool.tile([TT, 512], f32, tag="lg")
                if b < 2:
                    # vector ops may read only one PSUM operand each:
                    # lg = (pc + bias) then lg += pl
                    pc = s["pc"].pop(b)
                    nc.vector.tensor_tensor(lg, pc, b1s_sb[:, lo:lo + 512],
                                            Alu.add)
                    nc.vector.tensor_tensor(lg, lg, pl, Alu.add)
                else:
                    nc.vector.tensor_tensor(lg, pl, b1s_sb[:, lo:lo + 512],
                                            Alu.add)
                if b == 0:
                    nc.vector.tensor_scalar(s["d1"][:, 0:512], lg, 0.0, None,
                                            Alu.is_gt)
                elif b == 1:
                    nc.vector.tensor_scalar(s["d1"][:, 512:DEC_COLS],
                                            lg[:, 0:DEC_COLS - 512],
                                            0.0, None, Alu.is_gt)
                # leaf blocks: silu over the whole 512 (incl. pad cols; mk
                # zeroes the 2040:2048 tail later)
                nc.scalar.activation(s["ac"][:, lo:lo + 512], lg, Act.Silu,
                                     scale=1.0 / SC)

            def a_mask(j):
                """tree traversal mask + masked acts (fp16)."""
                s = st[j]
                d1, vv, ac = s["d1"], s["vv"], s["ac"]
                mk = mk_pool.tile([TT, NODES_PAD], f16, tag="mk")
                nc.vector.memset(vv[:, 0:8], 1.0)
                for d in range(DEPTH):
                    ld = 8 * (1 << d)
                    c0 = 8 * ((1 << d) - 1)
                    c1 = 8 * ((1 << (d + 1)) - 1)
                    vpar = vv[:, c0:c0 + ld].rearrange("p (i t) -> p i t", t=8)
                    dpar = d1[:, c0:c0 + ld].rearrange("p (i t) -> p i t", t=8)
                    kids = vv[:, c1:c1 + 2 * ld].rearrange(
                        "p (i two t) -> p i two t", two=2, t=8)
                    nc.vector.tensor_tensor(kids[:, :, 1, :], vpar, dpar,
                                            Alu.mult)
                    nc.vector.tensor_tensor(kids[:, :, 0, :], vpar,
                                            kids[:, :, 1, :], Alu.subtract)
                nc.vector.memset(mk[:, WIDTH:NODES_PAD], 0.0)
                # first 128 cols split out so stage_b's first transpose
                # can start before the rest of the masking finishes
                nc.vector.tensor_tensor(mk[:, 0:128], ac[:, 0:128],
                                        vv[:, 0:128], Alu.mult)
                nc.vector.tensor_tensor(mk[:, 128:1024], ac[:, 128:1024],
                                        vv[:, 128:1024], Alu.mult)
                nc.vector.tensor_tensor(mk[:, 1024:WIDTH], ac[:, 1024:WIDTH],
                                        vv[:, 1024:WIDTH], Alu.mult)
                s["mk"] = mk

            def a_full(j):
                a_init(j)
                a_main(j, 0)
                a_main(j, 1)
                a_corr(j, 0)
                a_epi(j, 0)
                a_corr(j, 1)
                a_epi(j, 1)
                a_leaf(j, 2)
                a_epi(j, 2)
                a_leaf(j, 3)
                a_epi(j, 3)
                a_mask(j)

            def stage_b(j):
                s = st.pop(j)
                mk = s["mk"]
                at = mk_pool.tile([128, C_CH, TT], f16, tag="at")
                c = 0
                for gsz in (1, 3, 4, 4, 4):
                    pt = pt_pool.tile([128, 512], f16)
                    for i in range(gsz):
                        nc.tensor.transpose(
                            pt[:, i * 128:(i + 1) * 128],
                            mk[:, (c + i) * 128:(c + i + 1) * 128], ident)
                    nc.scalar.copy(
                        at[:, c:c + gsz, :],
                        pt[:, :gsz * 128].rearrange("p (c t) -> p c t", t=TT))
                    c += gsz
                ys = out_pool.tile([TT, DIM], f32, tag="ys")
                hw = DIM // 2
                for h in range(2):
                    hs = slice(h * hw, (h + 1) * hw)
                    py = py_pool.tile([TT, hw], f32)
                    for c in range(C_CH):
                        nc.tensor.matmul(
                            py, lhsT=at[:, c, :], rhs=w2_sb[:, c, hs],
                            start=(c == 0), stop=(c == C_CH - 1))
                    nc.vector.tensor_copy(ys[:, hs], py)
                    nc.sync.dma_start(out=y[j * TT:(j + 1) * TT, hs],
                                      in_=ys[:, hs])

            # ---- ramp: DMAs emitted at point-of-need, 4 tiles of b0/b1
            # decision work interleaved with the weight stream, leaves and
            # GEMM2 deferred until their weights can have arrived ----
            nc.sync.dma_start(out=w1r_b0k0, in_=w1r[:, 0:1, 0:512])
            xs0a = wts.tile([128, 1, TT], f32r)
            nc.sync.dma_start(out=xs0a, in_=xs[:, 0, 0:1])
            xs0b = wts.tile([128, 3, TT], f32r)
            nc.sync.dma_start(out=xs0b, in_=xs[:, 0, 1:4])
            nc.sync.dma_start(out=w1r_b0a, in_=w1r[:, 1:4, 0:512])
            xs0c = wts.tile([128, 4, TT], f32r)
            nc.sync.dma_start(out=xs0c, in_=xs[:, 0, 4:8])
            nc.sync.dma_start(out=w1r_b0b, in_=w1r[:, 4:8, 0:512])
            xt_tiles[0] = (None, None, None)
            a_init(0)
            st[0]["xsat"] = lambda k: (
                xs0a[:, 0, :] if k == 0 else
                (xs0b[:, k - 1, :] if k < 4 else xs0c[:, k - 4, :]))
            a_main(0, 0)
            x80 = fetch_x8(0, nc.sync)
            st[0]["x"] = (None, x80, None)
            nc.sync.dma_start(out=w8p_a, in_=w8p[:, 0:4])
            nc.sync.dma_start(out=w8p_b, in_=w8p[:, 4:8])
            a_corr(0, 0)
            a_epi(0, 0)
            xs1 = fetch_xs(1, nc.sync)
            x81 = fetch_x8(1, nc.sync)
            xt_tiles[1] = (xs1, x81, None)
            a_init(1)
            a_main(1, 0)
            nc.sync.dma_start(out=w1r_b1, in_=w1r[:, :, 512:1024])
            a_main(0, 1)
            a_corr(0, 1)
            a_epi(0, 1)
            a_main(1, 1)
            a_corr(1, 0)
            a_epi(1, 0)
            a_corr(1, 1)
            a_epi(1, 1)
            xs2 = fetch_xs(2, nc.sync)
            x82 = fetch_x8(2, nc.sync)
            xt_tiles[2] = (xs2, x82, None)
            a_init(2)
            a_main(2, 0)
            a_main(2, 1)
            nc.sync.dma_start(out=w1h_b2, in_=w1h[:, :, 0:512])
            a_corr(2, 0)
            a_epi(2, 0)
            a_corr(2, 1)
            a_epi(2, 1)
            xs3 = fetch_xs(3, nc.sync)
            x83 = fetch_x8(3, nc.sync)
            xt_tiles[3] = (xs3, x83, None)
            a_init(3)
            a_main(3, 0)
            a_main(3, 1)
            nc.sync.dma_start(out=w1h_b3, in_=w1h[:, :, 512:1024])
            st[0]["x"] = (st[0]["x"][0], st[0]["x"][1], fetch_xh(0, nc.sync))
            st[1]["x"] = (st[1]["x"][0], st[1]["x"][1], fetch_xh(1, nc.sync))
            nc.sync.dma_start(out=w2_sb[:, 0:4, :], in_=w2[:, 0:4, :])
            nc.sync.dma_start(out=w2_sb[:, 4:8, :], in_=w2[:, 4:8, :])
            nc.sync.dma_start(out=w2_sb[:, 8:12, :], in_=w2[:, 8:12, :])
            nc.sync.dma_start(out=w2_sb[:, 12:16, :], in_=w2[:, 12:16, :])
            a_corr(3, 0)
            a_epi(3, 0)
            a_corr(3, 1)
            a_epi(3, 1)
            a_leaf(0, 2)
            a_epi(0, 2)
            st[2]["x"] = (st[2]["x"][0], st[2]["x"][1], fetch_xh(2, nc.sync))
            st[3]["x"] = (st[3]["x"][0], st[3]["x"][1], fetch_xh(3, nc.sync))
            a_leaf(0, 3)
            a_epi(0, 3)
            a_mask(0)
            a_leaf(1, 2)
            a_epi(1, 2)
            a_leaf(1, 3)
            a_epi(1, 3)
            a_mask(1)
            stage_b(0)
            a_leaf(2, 2)
            a_epi(2, 2)
            a_leaf(2, 3)
            a_epi(2, 3)
            a_mask(2)
            prefetch_xt(4)
            stage_b(1)
            a_leaf(3, 2)
            a_epi(3, 2)
            a_leaf(3, 3)
            a_epi(3, 3)
            a_mask(3)
            stage_b(2)
            # ---- steady state ----
            for j in range(4, NTILES):
                if j + 1 < NTILES:
                    prefetch_xt(j + 1)
                a_full(j)
                stage_b(j - 1)
            stage_b(NTILES - 1)

    nc.finalize()
    return nc


def _get_program():
    global _PROGRAM
    if _PROGRAM is None:
        _PROGRAM = _build_program()
    return _PROGRAM


def _round10(a):
    """Round fp32 to 10 explicit mantissa bits (RNE) — fp32r's internal
    operand precision, so the PE's own rounding becomes the identity."""
    bits = np.asarray(a, np.float32).view(np.uint32).copy()
    lsb = (bits >> np.uint32(13)) & np.uint32(1)
    bits = bits + np.uint32((1 << 12) - 1) + lsb
    bits &= np.uint32(~((1 << 13) - 1) & 0xFFFFFFFF)
    return bits.view(np.float32)


def kernel(oldx, W_in, b_in, W_out):
    from concourse.bass_utils import run_bass_kernel_spmd

    e4 = ml_dtypes.float8_e4m3
    oldx = np.asarray(oldx)
    W_in = np.asarray(W_in, dtype=np.float32)
    b_in = np.asarray(b_in, dtype=np.float32)
    W_out = np.asarray(W_out, dtype=np.float32)
    x = oldx.reshape(-1, DIM).astype(np.float32)          # [8192, 1024]

    # node-major column permutation: our col 8n+t  <-  ref col 255t+n
    i = np.arange(WIDTH)
    perm = 255 * (i % PAR) + (i // PAR)

    w1t = np.zeros((DIM, NODES_PAD), np.float32)
    w1t[:, :WIDTH] = W_in[perm, :].T
    w1q = _round10(w1t[:, :CORR_COLS])
    dw = (w1t[:, :CORR_COLS].astype(np.float64) - w1q).astype(np.float32)

    def chunk_w(a, dt, ncols):
        return np.ascontiguousarray(
            np.asarray(a, np.float32)[:, :ncols]
            .reshape(K_CH, 128, ncols).transpose(1, 0, 2)).astype(dt)

    w1r = chunk_w(w1q * np.float32(2 ** 9), np.float32, CORR_COLS)
    w8 = chunk_w(np.asarray((w1t[:, :CORR_COLS] * 64).astype(e4),
                            np.float32), np.float32, CORR_COLS)
    dw8 = chunk_w(np.asarray((dw * SC).astype(e4), np.float32),
                  np.float32, CORR_COLS)
    w8p = np.ascontiguousarray(
        np.stack([w8, dw8], axis=2)).astype(e4)           # [128,K,2,1024]
    # leaf weights bf16, pre-scaled 2^9 (exact) to match the global 2^17
    w1h = chunk_w(
        np.asarray((w1t[:, CORR_COLS:NODES_PAD] * np.float32(2 ** 9))
                   .astype(ml_dtypes.bfloat16), np.float32),
        ml_dtypes.bfloat16, 1024)
    b1sv = np.zeros(NODES_PAD, np.float32)
    b1sv[:WIDTH] = b_in[perm] * np.float32(SC)

    w2t = np.zeros((NODES_PAD, DIM), np.float32)
    w2t[:WIDTH] = W_out.T[perm, :]
    w2 = np.ascontiguousarray(
        w2t.astype(np.float16).reshape(C_CH, 128, DIM).transpose(1, 0, 2))

    in_maps = []
    for c in range(N_CORES):
        xc = x[c * TOK_PER_CORE:(c + 1) * TOK_PER_CORE]   # [1024, 1024]
        xT = np.ascontiguousarray(xc.T)                   # [dim, tok]
        xq = _round10(xT)
        dx = (xT.astype(np.float64) - xq).astype(np.float32)

        def lay(a):
            return np.ascontiguousarray(
                np.asarray(a, np.float32)
                .reshape(K_CH, 128, NTILES, TT).transpose(1, 2, 0, 3))

        xsv = lay(xq * np.float32(2 ** 8))
        dx8 = lay(np.asarray((dx * 2048).astype(e4), np.float32))
        x8 = lay(np.asarray(xT.astype(e4), np.float32))
        x8pv = np.ascontiguousarray(
            np.stack([dx8, x8], axis=3)).astype(e4)       # [128,NT,K,2,TT]
        xhv = lay(np.asarray((xT * np.float32(2 ** 8))
                             .astype(ml_dtypes.bfloat16), np.float32)
                  ).astype(ml_dtypes.bfloat16)
        in_maps.append({
            "xs": xsv, "x8p": x8pv, "xh": xhv,
            "w1r": w1r, "w8p": w8p, "w1h": w1h,
            "b1s": b1sv, "w2": w2,
        })

    nc = _get_program()
    res = run_bass_kernel_spmd(nc, in_maps, core_ids=list(range(N_CORES)))
    out = np.concatenate([res.results[c]["y"] for c in range(N_CORES)],
                         axis=0)
    return out.reshape(oldx.shape).astype(np.float32)


# revision 31
# speedup vs baseline: 1.1577x; 1.1577x over previous
"""Trainium2 Bass kernel for the FFF (fast feedforward / MoE-routing) module.

Math (per token x of dim 1024, PAR=8 trees of 255 nodes):
  logits = x @ W_in.T + b_in                      # [B, 2040]
  dec    = logits > 0
  acts   = silu(logits)
  dmap   = indicator of the 8 visited nodes per tree
  out    = (acts * dmap) @ W_out.T                # [B, 1024]

Strategy (8 NeuronCores, data-parallel over the 8192 tokens, 1024 each):
  - Decision region (cols 0..1024) in fp32r (TF32-like: PE rounds both
    operands to 10 explicit mantissa bits at ~bf16 speed).  Inputs are
    pre-rounded to 10 bits on the host so the fp32r pass is EXACT; the
    known residual dx@w + x@dw is computed via fp8e4 DoubleRow matmuls
    (2 planes/instr, 2x rate at M=128) into a separate PSUM and merged at
    the bias-add.  Main pass pre-scaled by 2^17 (x*2^8, w*2^9) to match
    the fp8 plane scales; silu uses the activation unit's input scale.
    Net logit error ~5e-6 rms -> ~0 decision flips vs fp32 reference.
  - Leaf region (cols 1024..2048) single-pass bf16 (acts only need ~1e-3).
  - dmap built level-by-level with strided vector ops in node-major
    column layout (col = 8*node + tree).
  - masked acts in fp16, transposed on the PE, GEMM2 in fp16.
  - DMAs emitted in need-order; tiles 0-1 are hand-interleaved with the
    weight-arrival schedule so the PE has work while weights stream.
"""

import numpy as np
import ml_dtypes

DIM = 1024
PAR = 8
DEPTH = 7
N_NODES = 255
WIDTH = PAR * N_NODES          # 2040
NODES_PAD = 2048
N_CORES = 8
TOK_PER_CORE = 1024
TT = 128
NTILES = TOK_PER_CORE // TT    # 8
K_CH = DIM // 128              # 8
C_CH = NODES_PAD // 128        # 16
DEC_COLS = 8 * 127             # 1016
CORR_COLS = 1024               # fp8-corrected region (blocks 0,1)
SC = float(2 ** 17)

_PROGRAM = None


def _build_program():
    import concourse.bacc as bacc
    import concourse.tile as tile
    from concourse import mybir
    from concourse.masks import make_identity
    import concourse.bass as bass

    f32 = mybir.dt.float32
    f32r = mybir.dt.float32r
    bf16 = mybir.dt.bfloat16
    f16 = mybir.dt.float16
    fp8e4 = mybir.dt.float8e4
    Alu = mybir.AluOpType
    Act = mybir.ActivationFunctionType
    DRM = mybir.MatmulPerfMode.DoubleRow

    nc = bacc.Bacc("TRN2", target_bir_lowering=False, debug=False,
                   num_devices=N_CORES)

    xs = nc.dram_tensor("xs", [128, NTILES, K_CH, TT], f32r,
                        kind="ExternalInput")
    x8p = nc.dram_tensor("x8p", [128, NTILES, K_CH, 2, TT], fp8e4,
                         kind="ExternalInput")
    xh = nc.dram_tensor("xh", [128, NTILES, K_CH, TT], bf16,
                        kind="ExternalInput")
    w1r = nc.dram_tensor("w1r", [128, K_CH, CORR_COLS], f32r,
                         kind="ExternalInput")
    w8p = nc.dram_tensor("w8p", [128, K_CH, 2, CORR_COLS], fp8e4,
                         kind="ExternalInput")
    w1h = nc.dram_tensor("w1h", [128, K_CH, 1024], bf16,
                         kind="ExternalInput")
    b1s = nc.dram_tensor("b1s", [NODES_PAD], f32, kind="ExternalInput")
    w2 = nc.dram_tensor("w2", [128, C_CH, DIM], f16, kind="ExternalInput")
    y = nc.dram_tensor("y", [TOK_PER_CORE, DIM], f32, kind="ExternalOutput")

    with tile.TileContext(nc) as tc:
        with (
            tc.tile_pool(name="wts", bufs=1) as wts,
            tc.tile_pool(name="xts", bufs=4) as xts,
            tc.tile_pool(name="lgs", bufs=2) as lgs_pool,
            tc.tile_pool(name="d1p", bufs=4) as d1_pool,
            tc.tile_pool(name="vvp", bufs=2) as vv_pool,
            tc.tile_pool(name="acp", bufs=4) as ac_pool,
            tc.tile_pool(name="mkp", bufs=2) as mk_pool,
            tc.tile_pool(name="out", bufs=2) as out_pool,
            tc.tile_pool(name="pl", bufs=3, space="PSUM") as pl_pool,
            tc.tile_pool(name="pc", bufs=2, space="PSUM") as pc_pool,
            tc.tile_pool(name="pt", bufs=2, space="PSUM") as pt_pool,
            tc.tile_pool(name="py", bufs=1, space="PSUM") as py_pool,
        ):
            # Weight tiles are split per DMA batch: the Tile framework
            # tracks dependencies at tile granularity, so a consumer waits
            # for ALL writes to its tile — separate tiles let the first
            # matmuls start as soon as their own bytes land.
            w1r_b0a = wts.tile([128, 2, 512], f32r)       # b0, k 0-1
            w1r_b0b = wts.tile([128, 6, 512], f32r)       # b0, k 2-7
            w1r_b1 = wts.tile([128, K_CH, 512], f32r)
            w8p_a = wts.tile([128, 4, 2, CORR_COLS], fp8e4)   # k 0-3
            w8p_b = wts.tile([128, 4, 2, CORR_COLS], fp8e4)   # k 4-7
            w1h_b2 = wts.tile([128, K_CH, 512], bf16)
            w1h_b3 = wts.tile([128, K_CH, 512], bf16)
            w2_sb = wts.tile([128, C_CH, DIM], f16)
            b1s_sb = wts.tile([128, NODES_PAD], f32)
            ident = wts.tile([128, 128], f16)

            def w1r_at(k, b):
                if b == 1:
                    return w1r_b1[:, k, :]
                return (w1r_b0a[:, k, :] if k < 2 else w1r_b0b[:, k - 2, :])

            def w8p_at(k, cols):
                t = w8p_a if k < 4 else w8p_b
                return t[:, k % 4, :, cols]

            def w1h_at(k, b):
                return (w1h_b2 if b == 2 else w1h_b3)[:, k, :]

            xt_tiles = {}

            def fetch_xs(j, eng):
                xsj = xts.tile([128, K_CH, TT], f32r, tag="xs")
                eng.dma_start(out=xsj, in_=xs[:, j])
                return xsj

            def fetch_x8(j, eng):
                x8j = xts.tile([128, K_CH, 2, TT], fp8e4, tag="x8")
                eng.dma_start(out=x8j, in_=x8p[:, j])
                return x8j

            def fetch_xh(j, eng):
                xhj = xts.tile([128, K_CH, TT], bf16, tag="xh")
                eng.dma_start(out=xhj, in_=xh[:, j])
                return xhj

            def prefetch_xt(j, eng=None):
                e = eng or nc.gpsimd
                xt_tiles[j] = (fetch_xs(j, e), fetch_x8(j, e), fetch_xh(j, e))

            # bias broadcast + identity off the Sync weight path, early
            b1_bcast = bass.AP(tensor=b1s, offset=0,
                               ap=[[0, 128], [1, NODES_PAD]])
            nc.gpsimd.dma_start(out=b1s_sb, in_=b1_bcast)
            make_identity(nc, ident)

            # per-tile stage-A state
            st = {}

            def a_init(j):
                if j not in xt_tiles:
                    prefetch_xt(j)
                d1 = d1_pool.tile([TT, DEC_COLS], f16, tag="d1")
                vv = vv_pool.tile([TT, WIDTH], f16, tag="vv")
                ac = ac_pool.tile([TT, NODES_PAD], f16, tag="ac")
                st[j] = {"x": xt_tiles.pop(j), "d1": d1, "vv": vv, "ac": ac,
                         "pl": {}, "pc": {}}

            def a_main(j, b):
                """fp32r main pass for block b (512 cols), group closed."""
                s = st[j]
                xsat = s.get("xsat") or (lambda k: s["x"][0][:, k, :])
                pl = pl_pool.tile([TT, 512], f32)
                for k in range(K_CH):
                    nc.tensor.matmul(pl, lhsT=xsat(k),
                                     rhs=w1r_at(k, b),
                                     start=(k == 0), stop=(k == K_CH - 1))
                s["pl"][b] = pl

            def a_corr(j, b):
                """fp8 DR correction for block b into its own PSUM."""
                s = st[j]
                x8j = s["x"][1]
                lo = b * 512
                pc = pc_pool.tile([TT, 512], f32)
                for q in range(2):
                    qs = slice(q * 256, (q + 1) * 256)
                    wq = slice(lo + q * 256, lo + (q + 1) * 256)
                    for k in range(K_CH):
                        nc.tensor.matmul(
                            pc[:, qs], lhsT=x8j[:, k, :, :],
                            rhs=w8p_at(k, wq),
                            start=(k == 0), stop=(k == K_CH - 1),
                            perf_mode=DRM, skip_group_check=True)
                s["pc"][b] = pc

            def a_leaf(j, b):
                """bf16 leaf pass for block b (2 or 3), group closed."""
                s = st[j]
                xhj = s["x"][2]
                pl = pl_pool.tile([TT, 512], f32)
                for k in range(K_CH):
                    nc.tensor.matmul(pl, lhsT=xhj[:, k, :],
                                     rhs=w1h_at(k, b),
                                     start=(k == 0), stop=(k == K_CH - 1))
                s["pl"][b] = pl

            def a_epi(j, b):
                """bias (+corr) add, decisions, silu for block b."""
                s = st[j]
                lo = b * 512
                pl = s["pl"].pop(b)
                lg = lgs_pool.tile([TT, 512], f32, tag="lg")
                if b < 2:
                    # vector ops may read only one PSUM operand each:
                    # lg = (pc + bias) then lg += pl
                    pc = s["pc"].pop(b)
                    nc.vector.tensor_tensor(lg, pc, b1s_sb[:, lo:lo + 512],
                                            Alu.add)
                    nc.vector.tensor_tensor(lg, lg, pl, Alu.add)
                else:
                    nc.vector.tensor_tensor(lg, pl, b1s_sb[:, lo:lo + 512],
                                            Alu.add)
                if b == 0:
                    nc.vector.tensor_scalar(s["d1"][:, 0:512], lg, 0.0, None,
                                            Alu.is_gt)
                elif b == 1:
                    nc.vector.tensor_scalar(s["d1"][:, 512:DEC_COLS],
                                            lg[:, 0:DEC_COLS - 512],
                                            0.0, None, Alu.is_gt)
                # leaf blocks: silu over the whole 512 (incl. pad cols; mk
                # zeroes the 2040:2048 tail later)
                nc.scalar.activation(s["ac"][:, lo:lo + 512], lg, Act.Silu,
                                     scale=1.0 / SC)

            def a_mask(j):
                """tree traversal mask + masked acts (fp16)."""
                s = st[j]
                d1, vv, ac = s["d1"], s["vv"], s["ac"]
                mk = mk_pool.tile([TT, NODES_PAD], f16, tag="mk")
                nc.vector.memset(vv[:, 0:8], 1.0)
                for d in range(DEPTH):
                    ld = 8 * (1 << d)
                    c0 = 8 * ((1 << d) - 1)
                    c1 = 8 * ((1 << (d + 1)) - 1)
                    vpar = vv[:, c0:c0 + ld].rearrange("p (i t) -> p i t", t=8)
                    dpar = d1[:, c0:c0 + ld].rearrange("p (i t) -> p i t", t=8)
                    kids = vv[:, c1:c1 + 2 * ld].rearrange(
                        "p (i two t) -> p i two t", two=2, t=8)
                    nc.vector.tensor_tensor(kids[:, :, 1, :], vpar, dpar,
                                            Alu.mult)
                    nc.vector.tensor_tensor(kids[:, :, 0, :], vpar,
                                            kids[:, :, 1, :], Alu.subtract)
                nc.vector.memset(mk[:, WIDTH:NODES_PAD], 0.0)
                # first 128 cols split out so stage_b's first transpose
                # can start before the rest of the masking finishes
                nc.vector.tensor_tensor(mk[:, 0:128], ac[:, 0:128],
                                        vv[:, 0:128], Alu.mult)
                nc.vector.tensor_tensor(mk[:, 128:1024], ac[:, 128:1024],
                                        vv[:, 128:1024], Alu.mult)
                nc.vector.tensor_tensor(mk[:, 1024:WIDTH], ac[:, 1024:WIDTH],
                                        vv[:, 1024:WIDTH], Alu.mult)
                s["mk"] = mk

            def a_full(j):
                a_init(j)
                a_main(j, 0)
                a_main(j, 1)
                a_corr(j, 0)
                a_epi(j, 0)
                a_corr(j, 1)
                a_epi(j, 1)
                a_leaf(j, 2)
                a_epi(j, 2)
                a_leaf(j, 3)
                a_epi(j, 3)
                a_mask(j)

            def stage_b(j):
                s = st.pop(j)
                mk = s["mk"]
                at = mk_pool.tile([128, C_CH, TT], f16, tag="at")
                c = 0
                for gsz in (1, 3, 4, 4, 4):
                    pt = pt_pool.tile([128, 512], f16)
                    for i in range(gsz):
                        nc.tensor.transpose(
                            pt[:, i * 128:(i + 1) * 128],
                            mk[:, (c + i) * 128:(c + i + 1) * 128], ident)
                    nc.scalar.copy(
                        at[:, c:c + gsz, :],
                        pt[:, :gsz * 128].rearrange("p (c t) -> p c t", t=TT))
                    c += gsz
                ys = out_pool.tile([TT, DIM], f32, tag="ys")
                hw = DIM // 2
                for h in range(2):
                    hs = slice(h * hw, (h + 1) * hw)
                    py = py_pool.tile([TT, hw], f32)
                    for c in range(C_CH):
                        nc.tensor.matmul(
                            py, lhsT=at[:, c, :], rhs=w2_sb[:, c, hs],
                            start=(c == 0), stop=(c == C_CH - 1))
                    nc.vector.tensor_copy(ys[:, hs], py)
                    nc.sync.dma_start(out=y[j * TT:(j + 1) * TT, hs],
                                      in_=ys[:, hs])

            # ---- ramp: DMAs emitted at point-of-need, 4 tiles of b0/b1
            # decision work interleaved with the weight stream, leaves and
            # GEMM2 deferred until their weights can have arrived ----
            nc.sync.dma_start(out=w1r_b0a, in_=w1r[:, 0:2, 0:512])
            xs0 = fetch_xs(0, nc.sync)
            nc.sync.dma_start(out=w1r_b0b, in_=w1r[:, 2:8, 0:512])
            xt_tiles[0] = (xs0, None, None)
            a_init(0)
            a_main(0, 0)
            x80 = fetch_x8(0, nc.sync)
            st[0]["x"] = (xs0, x80, None)
            nc.sync.dma_start(out=w8p_a, in_=w8p[:, 0:4])
            nc.sync.dma_start(out=w8p_b, in_=w8p[:, 4:8])
            a_corr(0, 0)
            a_epi(0, 0)
            xs1 = fetch_xs(1, nc.sync)
            x81 = fetch_x8(1, nc.sync)
            xt_tiles[1] = (xs1, x81, None)
            a_init(1)
            a_main(1, 0)
            nc.sync.dma_start(out=w1r_b1, in_=w1r[:, :, 512:1024])
            a_main(0, 1)
            a_corr(0, 1)
            a_epi(0, 1)
            a_main(1, 1)
            a_corr(1, 0)
            a_epi(1, 0)
            a_corr(1, 1)
            a_epi(1, 1)
            xs2 = fetch_xs(2, nc.sync)
            x82 = fetch_x8(2, nc.sync)
            xt_tiles[2] = (xs2, x82, None)
            a_init(2)
            a_main(2, 0)
            a_main(2, 1)
            nc.sync.dma_start(out=w1h_b2, in_=w1h[:, :, 0:512])
            a_corr(2, 0)
            a_epi(2, 0)
            a_corr(2, 1)
            a_epi(2, 1)
            xs3 = fetch_xs(3, nc.sync)
            x83 = fetch_x8(3, nc.sync)
            xt_tiles[3] = (xs3, x83, None)
            a_init(3)
            a_main(3, 0)
            a_main(3, 1)
            nc.sync.dma_start(out=w1h_b3, in_=w1h[:, :, 512:1024])
            st[0]["x"] = (st[0]["x"][0], st[0]["x"][1], fetch_xh(0, nc.sync))
            st[1]["x"] = (st[1]["x"][0], st[1]["x"][1], fetch_xh(1, nc.sync))
            nc.sync.dma_start(out=w2_sb[:, 0:4, :], in_=w2[:, 0:4, :])
            nc.sync.dma_start(out=w2_sb[:, 4:8, :], in_=w2[:, 4:8, :])
            nc.sync.dma_start(out=w2_sb[:, 8:12, :], in_=w2[:, 8:12, :])
            nc.sync.dma_start(out=w2_sb[:, 12:16, :], in_=w2[:, 12:16, :])
            a_corr(3, 0)
            a_epi(3, 0)
            a_corr(3, 1)
            a_epi(3, 1)
            a_leaf(0, 2)
            a_epi(0, 2)
            st[2]["x"] = (st[2]["x"][0], st[2]["x"][1], fetch_xh(2, nc.sync))
            st[3]["x"] = (st[3]["x"][0], st[3]["x"][1], fetch_xh(3, nc.sync))
            a_leaf(0, 3)
            a_epi(0, 3)
            a_mask(0)
            a_leaf(1, 2)
            a_epi(1, 2)
            a_leaf(1, 3)
            a_epi(1, 3)
            a_mask(1)
            stage_b(0)
            a_leaf(2, 2)
            a_epi(2, 2)
            a_leaf(2, 3)
            a_epi(2, 3)
            a_mask(2)
            prefetch_xt(4)
            stage_b(1)
            a_leaf(3, 2)
            a_epi(3, 2)
            a_leaf(3, 3)
            a_epi(3, 3)
            a_mask(3)
            stage_b(2)
            # ---- steady state ----
            for j in range(4, NTILES):
                if j + 1 < NTILES:
                    prefetch_xt(j + 1)
                a_full(j)
                stage_b(j - 1)
            stage_b(NTILES - 1)

    nc.finalize()
    return nc


def _get_program():
    global _PROGRAM
    if _PROGRAM is None:
        _PROGRAM = _build_program()
    return _PROGRAM


def _round10(a):
    """Round fp32 to 10 explicit mantissa bits (RNE) — fp32r's internal
    operand precision, so the PE's own rounding becomes the identity."""
    bits = np.asarray(a, np.float32).view(np.uint32).copy()
    lsb = (bits >> np.uint32(13)) & np.uint32(1)
    bits = bits + np.uint32((1 << 12) - 1) + lsb
    bits &= np.uint32(~((1 << 13) - 1) & 0xFFFFFFFF)
    return bits.view(np.float32)


def kernel(oldx, W_in, b_in, W_out):
    from concourse.bass_utils import run_bass_kernel_spmd

    e4 = ml_dtypes.float8_e4m3
    oldx = np.asarray(oldx)
    W_in = np.asarray(W_in, dtype=np.float32)
    b_in = np.asarray(b_in, dtype=np.float32)
    W_out = np.asarray(W_out, dtype=np.float32)
    x = oldx.reshape(-1, DIM).astype(np.float32)          # [8192, 1024]

    # node-major column permutation: our col 8n+t  <-  ref col 255t+n
    i = np.arange(WIDTH)
    perm = 255 * (i % PAR) + (i // PAR)

    w1t = np.zeros((DIM, NODES_PAD), np.float32)
    w1t[:, :WIDTH] = W_in[perm, :].T
    w1q = _round10(w1t[:, :CORR_COLS])
    dw = (w1t[:, :CORR_COLS].astype(np.float64) - w1q).astype(np.float32)

    def chunk_w(a, dt, ncols):
        return np.ascontiguousarray(
            np.asarray(a, np.float32)[:, :ncols]
            .reshape(K_CH, 128, ncols).transpose(1, 0, 2)).astype(dt)

    w1r = chunk_w(w1q * np.float32(2 ** 9), np.float32, CORR_COLS)
    w8 = chunk_w(np.asarray((w1t[:, :CORR_COLS] * 64).astype(e4),
                            np.float32), np.float32, CORR_COLS)
    dw8 = chunk_w(np.asarray((dw * SC).astype(e4), np.float32),
                  np.float32, CORR_COLS)
    w8p = np.ascontiguousarray(
        np.stack([w8, dw8], axis=2)).astype(e4)           # [128,K,2,1024]
    # leaf weights bf16, pre-scaled 2^9 (exact) to match the global 2^17
    w1h = chunk_w(
        np.asarray((w1t[:, CORR_COLS:NODES_PAD] * np.float32(2 ** 9))
                   .astype(ml_dtypes.bfloat16), np.float32),
        ml_dtypes.bfloat16, 1024)
    b1sv = np.zeros(NODES_PAD, np.float32)
    b1sv[:WIDTH] = b_in[perm] * np.float32(SC)

    w2t = np.zeros((NODES_PAD, DIM), np.float32)
    w2t[:WIDTH] = W_out.T[perm, :]
    w2 = np.ascontiguousarray(
        w2t.astype(np.float16).reshape(C_CH, 128, DIM).transpose(1, 0, 2))

    in_maps = []
    for c in range(N_CORES):
        xc = x[c * TOK_PER_CORE:(c + 1) * TOK_PER_CORE]   # [1024, 1024]
        xT = np.ascontiguousarray(xc.T)                   # [dim, tok]
        xq = _round10(xT)
        dx = (xT.astype(np.float64) - xq).astype(np.float32)

        def lay(a):
            return np.ascontiguousarray(
                np.asarray(a, np.float32)
                .reshape(K_CH, 128, NTILES, TT).transpose(1, 2, 0, 3))

        xsv = lay(xq * np.float32(2 ** 8))
        dx8 = lay(np.asarray((dx * 2048).astype(e4), np.float32))
        x8 = lay(np.asarray(xT.astype(e4), np.float32))
        x8pv = np.ascontiguousarray(
            np.stack([dx8, x8], axis=3)).astype(e4)       # [128,NT,K,2,TT]
        xhv = lay(np.asarray((xT * np.float32(2 ** 8))
                             .astype(ml_dtypes.bfloat16), np.float32)
                  ).astype(ml_dtypes.bfloat16)
        in_maps.append({
            "xs": xsv, "x8p": x8pv, "xh": xhv,
            "w1r": w1r, "w8p": w8p, "w1h": w1h,
            "b1s": b1sv, "w2": w2,
        })

    nc = _get_program()
    res = run_bass_kernel_spmd(nc, in_maps, core_ids=list(range(N_CORES)))
    out = np.concatenate([res.results[c]["y"] for c in range(N_CORES)],
                         axis=0)
    return out.reshape(oldx.shape).astype(np.float32)


# revision 33
# speedup vs baseline: 1.1619x; 1.0036x over previous
"""Trainium2 Bass kernel for the FFF (fast feedforward / MoE-routing) module.

Math (per token x of dim 1024, PAR=8 trees of 255 nodes):
  logits = x @ W_in.T + b_in                      # [B, 2040]
  dec    = logits > 0
  acts   = silu(logits)
  dmap   = indicator of the 8 visited nodes per tree
  out    = (acts * dmap) @ W_out.T                # [B, 1024]

Strategy (8 NeuronCores, data-parallel over the 8192 tokens, 1024 each):
  - Decision region (cols 0..1024) in fp32r (TF32-like: PE rounds both
    operands to 10 explicit mantissa bits at ~bf16 speed).  Inputs are
    pre-rounded to 10 bits on the host so the fp32r pass is EXACT; the
    known residual dx@w + x@dw is computed via fp8e4 DoubleRow matmuls
    (2 planes/instr, 2x rate at M=128) into a separate PSUM and merged at
    the bias-add.  Main pass pre-scaled by 2^17 (x*2^8, w*2^9) to match
    the fp8 plane scales; silu uses the activation unit's input scale.
    Net logit error ~5e-6 rms -> ~0 decision flips vs fp32 reference.
  - Leaf region (cols 1024..2048) single-pass bf16 (acts only need ~1e-3).
  - dmap built level-by-level with strided vector ops in node-major
    column layout (col = 8*node + tree).
  - masked acts in fp16, transposed on the PE, GEMM2 in fp16.
  - DMAs emitted in need-order; tiles 0-1 are hand-interleaved with the
    weight-arrival schedule so the PE has work while weights stream.
"""

import numpy as np
import ml_dtypes

DIM = 1024
PAR = 8
DEPTH = 7
N_NODES = 255
WIDTH = PAR * N_NODES          # 2040
NODES_PAD = 2048
N_CORES = 8
TOK_PER_CORE = 1024
TT = 128
NTILES = TOK_PER_CORE // TT    # 8
K_CH = DIM // 128              # 8
C_CH = NODES_PAD // 128        # 16
DEC_COLS = 8 * 127             # 1016
CORR_COLS = 1024               # fp8-corrected region (blocks 0,1)
SC = float(2 ** 17)

_PROGRAM = None


def _build_program():
    import concourse.bacc as bacc
    import concourse.tile as tile
    from concourse import mybir
    from concourse.masks import make_identity
    import concourse.bass as bass

    f32 = mybir.dt.float32
    f32r = mybir.dt.float32r
    bf16 = mybir.dt.bfloat16
    f16 = mybir.dt.float16
    fp8e4 = mybir.dt.float8e4
    Alu = mybir.AluOpType
    Act = mybir.ActivationFunctionType
    DRM = mybir.MatmulPerfMode.DoubleRow

    nc = bacc.Bacc("TRN2", target_bir_lowering=False, debug=False,
                   num_devices=N_CORES)

    xs = nc.dram_tensor("xs", [128, NTILES, K_CH, TT], f32r,
                        kind="ExternalInput")
    x8p = nc.dram_tensor("x8p", [128, NTILES, K_CH, 2, TT], fp8e4,
                         kind="ExternalInput")
    xh = nc.dram_tensor("xh", [128, NTILES, K_CH, TT], bf16,
                        kind="ExternalInput")
    w1r = nc.dram_tensor("w1r", [128, K_CH, CORR_COLS], f32r,
                         kind="ExternalInput")
    w8p = nc.dram_tensor("w8p", [128, K_CH, 2, CORR_COLS], fp8e4,
                         kind="ExternalInput")
    w1h = nc.dram_tensor("w1h", [128, K_CH, 1024], bf16,
                         kind="ExternalInput")
    b1s = nc.dram_tensor("b1s", [NODES_PAD], f32, kind="ExternalInput")
    w2 = nc.dram_tensor("w2", [128, C_CH, DIM], f16, kind="ExternalInput")
    y = nc.dram_tensor("y", [TOK_PER_CORE, DIM], f32, kind="ExternalOutput")

    with tile.TileContext(nc) as tc:
        with (
            tc.tile_pool(name="wts", bufs=1) as wts,
            tc.tile_pool(name="xts", bufs=4) as xts,
            tc.tile_pool(name="lgs", bufs=2) as lgs_pool,
            tc.tile_pool(name="d1p", bufs=4) as d1_pool,
            tc.tile_pool(name="vvp", bufs=2) as vv_pool,
            tc.tile_pool(name="acp", bufs=4) as ac_pool,
            tc.tile_pool(name="mkp", bufs=2) as mk_pool,
            tc.tile_pool(name="out", bufs=2) as out_pool,
            tc.tile_pool(name="pl", bufs=3, space="PSUM") as pl_pool,
            tc.tile_pool(name="pc", bufs=2, space="PSUM") as pc_pool,
            tc.tile_pool(name="pt", bufs=2, space="PSUM") as pt_pool,
            tc.tile_pool(name="py", bufs=1, space="PSUM") as py_pool,
        ):
            # Weight tiles are split per DMA batch: the Tile framework
            # tracks dependencies at tile granularity, so a consumer waits
            # for ALL writes to its tile — separate tiles let the first
            # matmuls start as soon as their own bytes land.
            w1r_b0a = wts.tile([128, 2, 512], f32r)       # b0, k 0-1
            w1r_b0b = wts.tile([128, 6, 512], f32r)       # b0, k 2-7
            w1r_b1 = wts.tile([128, K_CH, 512], f32r)
            w8p_a = wts.tile([128, 4, 2, CORR_COLS], fp8e4)   # k 0-3
            w8p_b = wts.tile([128, 4, 2, CORR_COLS], fp8e4)   # k 4-7
            w1h_b2 = wts.tile([128, K_CH, 512], bf16)
            w1h_b3 = wts.tile([128, K_CH, 512], bf16)
            w2_sb = wts.tile([128, C_CH, DIM], f16)
            b1s_sb = wts.tile([128, NODES_PAD], f32)
            ident = wts.tile([128, 128], f16)

            def w1r_at(k, b):
                if b == 1:
                    return w1r_b1[:, k, :]
                return (w1r_b0a[:, k, :] if k < 2 else w1r_b0b[:, k - 2, :])

            def w8p_at(k, cols):
                t = w8p_a if k < 4 else w8p_b
                return t[:, k % 4, :, cols]

            def w1h_at(k, b):
                return (w1h_b2 if b == 2 else w1h_b3)[:, k, :]

            xt_tiles = {}

            def fetch_xs(j, eng):
                xsj = xts.tile([128, K_CH, TT], f32r, tag="xs")
                eng.dma_start(out=xsj, in_=xs[:, j])
                return xsj

            def fetch_x8(j, eng):
                x8j = xts.tile([128, K_CH, 2, TT], fp8e4, tag="x8")
                eng.dma_start(out=x8j, in_=x8p[:, j])
                return x8j

            def fetch_xh(j, eng):
                xhj = xts.tile([128, K_CH, TT], bf16, tag="xh")
                eng.dma_start(out=xhj, in_=xh[:, j])
                return xhj

            def prefetch_xt(j, eng=None):
                e = eng or nc.gpsimd
                xt_tiles[j] = (fetch_xs(j, e), fetch_x8(j, e), fetch_xh(j, e))

            # bias broadcast + identity off the Sync weight path, early
            b1_bcast = bass.AP(tensor=b1s, offset=0,
                               ap=[[0, 128], [1, NODES_PAD]])
            nc.gpsimd.dma_start(out=b1s_sb, in_=b1_bcast)
            make_identity(nc, ident)

            # per-tile stage-A state
            st = {}

            def a_init(j):
                if j not in xt_tiles:
                    prefetch_xt(j)
                d1 = d1_pool.tile([TT, DEC_COLS], f16, tag="d1")
                vv = vv_pool.tile([TT, WIDTH], f16, tag="vv")
                ac = ac_pool.tile([TT, NODES_PAD], f16, tag="ac")
                st[j] = {"x": xt_tiles.pop(j), "d1": d1, "vv": vv, "ac": ac,
                         "pl": {}, "pc": {}}

            def a_main(j, b):
                """fp32r main pass for block b (512 cols), group closed."""
                s = st[j]
                xsat = s.get("xsat") or (lambda k: s["x"][0][:, k, :])
                pl = pl_pool.tile([TT, 512], f32)
                for k in range(K_CH):
                    nc.tensor.matmul(pl, lhsT=xsat(k),
                                     rhs=w1r_at(k, b),
                                     start=(k == 0), stop=(k == K_CH - 1))
                s["pl"][b] = pl

            def a_corr(j, b):
                """fp8 DR correction for block b into its own PSUM."""
                s = st[j]
                x8j = s["x"][1]
                lo = b * 512
                pc = pc_pool.tile([TT, 512], f32)
                for q in range(2):
                    qs = slice(q * 256, (q + 1) * 256)
                    wq = slice(lo + q * 256, lo + (q + 1) * 256)
                    for k in range(K_CH):
                        nc.tensor.matmul(
                            pc[:, qs], lhsT=x8j[:, k, :, :],
                            rhs=w8p_at(k, wq),
                            start=(k == 0), stop=(k == K_CH - 1),
                            perf_mode=DRM, skip_group_check=True)
                s["pc"][b] = pc

            def a_leaf(j, b):
                """bf16 leaf pass for block b (2 or 3), group closed."""
                s = st[j]
                xhj = s["x"][2]
                pl = pl_pool.tile([TT, 512], f32)
                for k in range(K_CH):
                    nc.tensor.matmul(pl, lhsT=xhj[:, k, :],
                                     rhs=w1h_at(k, b),
                                     start=(k == 0), stop=(k == K_CH - 1))
                s["pl"][b] = pl

            def a_epi(j, b):
                """bias (+corr) add, decisions, silu for block b."""
                s = st[j]
                lo = b * 512
                pl = s["pl"].pop(b)
                lg = lgs_pool.tile([TT, 512], f32, tag="lg")
                if b in s["pc"]:
                    # vector ops may read only one PSUM operand each:
                    # lg = (pc + bias) then lg += pl
                    pc = s["pc"].pop(b)
                    nc.vector.tensor_tensor(lg, pc, b1s_sb[:, lo:lo + 512],
                                            Alu.add)
                    nc.vector.tensor_tensor(lg, lg, pl, Alu.add)
                else:
                    nc.vector.tensor_tensor(lg, pl, b1s_sb[:, lo:lo + 512],
                                            Alu.add)
                if b == 0:
                    nc.vector.tensor_scalar(s["d1"][:, 0:512], lg, 0.0, None,
                                            Alu.is_gt)
                elif b == 1:
                    nc.vector.tensor_scalar(s["d1"][:, 512:DEC_COLS],
                                            lg[:, 0:DEC_COLS - 512],
                                            0.0, None, Alu.is_gt)
                # leaf blocks: silu over the whole 512 (incl. pad cols; mk
                # zeroes the 2040:2048 tail later)
                nc.scalar.activation(s["ac"][:, lo:lo + 512], lg, Act.Silu,
                                     scale=1.0 / SC)

            def a_mask(j):
                """tree traversal mask + masked acts (fp16)."""
                s = st[j]
                d1, vv, ac = s["d1"], s["vv"], s["ac"]
                mk = mk_pool.tile([TT, NODES_PAD], f16, tag="mk")
                nc.vector.memset(vv[:, 0:8], 1.0)
                for d in range(DEPTH):
                    ld = 8 * (1 << d)
                    c0 = 8 * ((1 << d) - 1)
                    c1 = 8 * ((1 << (d + 1)) - 1)
                    vpar = vv[:, c0:c0 + ld].rearrange("p (i t) -> p i t", t=8)
                    dpar = d1[:, c0:c0 + ld].rearrange("p (i t) -> p i t", t=8)
                    kids = vv[:, c1:c1 + 2 * ld].rearrange(
                        "p (i two t) -> p i two t", two=2, t=8)
                    nc.vector.tensor_tensor(kids[:, :, 1, :], vpar, dpar,
                                            Alu.mult)
                    nc.vector.tensor_tensor(kids[:, :, 0, :], vpar,
                                            kids[:, :, 1, :], Alu.subtract)
                nc.vector.memset(mk[:, WIDTH:NODES_PAD], 0.0)
                # first 128 cols split out so stage_b's first transpose
                # can start before the rest of the masking finishes
                nc.vector.tensor_tensor(mk[:, 0:128], ac[:, 0:128],
                                        vv[:, 0:128], Alu.mult)
                nc.vector.tensor_tensor(mk[:, 128:1024], ac[:, 128:1024],
                                        vv[:, 128:1024], Alu.mult)
                nc.vector.tensor_tensor(mk[:, 1024:WIDTH], ac[:, 1024:WIDTH],
                                        vv[:, 1024:WIDTH], Alu.mult)
                s["mk"] = mk

            def a_fused(j, b):
                """steady state: fp32r + DR corrections in ONE PSUM group
                (weights resident, no need for the ramp's split-PSUM)."""
                s = st[j]
                xsat = s.get("xsat") or (lambda k: s["x"][0][:, k, :])
                x8j = s["x"][1]
                lo = b * 512
                pl = pl_pool.tile([TT, 512], f32)
                for k in range(K_CH):
                    nc.tensor.matmul(pl, lhsT=xsat(k), rhs=w1r_at(k, b),
                                     start=(k == 0), stop=False)
                for q in range(2):
                    qs = slice(q * 256, (q + 1) * 256)
                    wq = slice(lo + q * 256, lo + (q + 1) * 256)
                    for k in range(K_CH):
                        nc.tensor.matmul(
                            pl[:, qs], lhsT=x8j[:, k, :, :],
                            rhs=w8p_at(k, wq),
                            start=False, stop=(q == 1 and k == K_CH - 1),
                            perf_mode=DRM, skip_group_check=True)
                s["pl"][b] = pl

            def a_full(j):
                a_init(j)
                a_fused(j, 0)
                a_epi(j, 0)
                a_fused(j, 1)
                a_epi(j, 1)
                a_leaf(j, 2)
                a_epi(j, 2)
                a_leaf(j, 3)
                a_epi(j, 3)
                a_mask(j)

            def stage_b(j):
                s = st.pop(j)
                mk = s["mk"]
                at = mk_pool.tile([128, C_CH, TT], f16, tag="at")
                c = 0
                for gsz in (1, 3, 4, 4, 4):
                    pt = pt_pool.tile([128, 512], f16)
                    for i in range(gsz):
                        nc.tensor.transpose(
                            pt[:, i * 128:(i + 1) * 128],
                            mk[:, (c + i) * 128:(c + i + 1) * 128], ident)
                    nc.scalar.copy(
                        at[:, c:c + gsz, :],
                        pt[:, :gsz * 128].rearrange("p (c t) -> p c t", t=TT))
                    c += gsz
                ys = out_pool.tile([TT, DIM], f32, tag="ys")
                hw = DIM // 2
                for h in range(2):
                    hs = slice(h * hw, (h + 1) * hw)
                    py = py_pool.tile([TT, hw], f32)
                    for c in range(C_CH):
                        nc.tensor.matmul(
                            py, lhsT=at[:, c, :], rhs=w2_sb[:, c, hs],
                            start=(c == 0), stop=(c == C_CH - 1))
                    nc.vector.tensor_copy(ys[:, hs], py)
                    nc.sync.dma_start(out=y[j * TT:(j + 1) * TT, hs],
                                      in_=ys[:, hs])

            # ---- ramp: DMAs emitted at point-of-need, 4 tiles of b0/b1
            # decision work interleaved with the weight stream, leaves and
            # GEMM2 deferred until their weights can have arrived ----
            nc.sync.dma_start(out=w1r_b0a, in_=w1r[:, 0:2, 0:512])
            xs0 = fetch_xs(0, nc.sync)
            nc.sync.dma_start(out=w1r_b0b, in_=w1r[:, 2:8, 0:512])
            xt_tiles[0] = (xs0, None, None)
            a_init(0)
            a_main(0, 0)
            x80 = fetch_x8(0, nc.sync)
            st[0]["x"] = (xs0, x80, None)
            nc.sync.dma_start(out=w8p_a, in_=w8p[:, 0:4])
            nc.sync.dma_start(out=w8p_b, in_=w8p[:, 4:8])
            a_corr(0, 0)
            a_epi(0, 0)
            xs1 = fetch_xs(1, nc.sync)
            x81 = fetch_x8(1, nc.sync)
            xt_tiles[1] = (xs1, x81, None)
            a_init(1)
            a_main(1, 0)
            nc.sync.dma_start(out=w1r_b1, in_=w1r[:, :, 512:1024])
            a_main(0, 1)
            a_corr(0, 1)
            a_epi(0, 1)
            a_main(1, 1)
            a_corr(1, 0)
            a_epi(1, 0)
            a_corr(1, 1)
            a_epi(1, 1)
            xs2 = fetch_xs(2, nc.sync)
            x82 = fetch_x8(2, nc.sync)
            xt_tiles[2] = (xs2, x82, None)
            a_init(2)
            a_main(2, 0)
            a_main(2, 1)
            nc.sync.dma_start(out=w1h_b2, in_=w1h[:, :, 0:512])
            a_corr(2, 0)
            a_epi(2, 0)
            a_corr(2, 1)
            a_epi(2, 1)
            xs3 = fetch_xs(3, nc.sync)
            x83 = fetch_x8(3, nc.sync)
            xt_tiles[3] = (xs3, x83, None)
            a_init(3)
            a_main(3, 0)
            a_main(3, 1)
            nc.sync.dma_start(out=w1h_b3, in_=w1h[:, :, 512:1024])
            st[0]["x"] = (st[0]["x"][0], st[0]["x"][1], fetch_xh(0, nc.sync))
            st[1]["x"] = (st[1]["x"][0], st[1]["x"][1], fetch_xh(1, nc.sync))
            nc.sync.dma_start(out=w2_sb[:, 0:4, :], in_=w2[:, 0:4, :])
            nc.sync.dma_start(out=w2_sb[:, 4:8, :], in_=w2[:, 4:8, :])
            nc.sync.dma_start(out=w2_sb[:, 8:12, :], in_=w2[:, 8:12, :])
            nc.sync.dma_start(out=w2_sb[:, 12:16, :], in_=w2[:, 12:16, :])
            a_corr(3, 0)
            a_epi(3, 0)
            a_corr(3, 1)
            a_epi(3, 1)
            a_leaf(0, 2)
            a_epi(0, 2)
            st[2]["x"] = (st[2]["x"][0], st[2]["x"][1], fetch_xh(2, nc.sync))
            st[3]["x"] = (st[3]["x"][0], st[3]["x"][1], fetch_xh(3, nc.sync))
            a_leaf(0, 3)
            a_epi(0, 3)
            a_mask(0)
            a_leaf(1, 2)
            a_epi(1, 2)
            a_leaf(1, 3)
            a_epi(1, 3)
            a_mask(1)
            stage_b(0)
            a_leaf(2, 2)
            a_epi(2, 2)
            a_leaf(2, 3)
            a_epi(2, 3)
            a_mask(2)
            prefetch_xt(4)
            stage_b(1)
            a_leaf(3, 2)
            a_epi(3, 2)
            a_leaf(3, 3)
            a_epi(3, 3)
            a_mask(3)
            stage_b(2)
            # ---- steady state ----
            for j in range(4, NTILES):
                if j + 1 < NTILES:
                    prefetch_xt(j + 1)
                a_full(j)
                stage_b(j - 1)
            stage_b(NTILES - 1)

    nc.finalize()
    return nc


def _get_program():
    global _PROGRAM
    if _PROGRAM is None:
        _PROGRAM = _build_program()
    return _PROGRAM


def _round10(a):
    """Round fp32 to 10 explicit mantissa bits (RNE) — fp32r's internal
    operand precision, so the PE's own rounding becomes the identity."""
    bits = np.asarray(a, np.float32).view(np.uint32).copy()
    lsb = (bits >> np.uint32(13)) & np.uint32(1)
    bits = bits + np.uint32((1 << 12) - 1) + lsb
    bits &= np.uint32(~((1 << 13) - 1) & 0xFFFFFFFF)
    return bits.view(np.float32)


def kernel(oldx, W_in, b_in, W_out):
    from concourse.bass_utils import run_bass_kernel_spmd

    e4 = ml_dtypes.float8_e4m3
    oldx = np.asarray(oldx)
    W_in = np.asarray(W_in, dtype=np.float32)
    b_in = np.asarray(b_in, dtype=np.float32)
    W_out = np.asarray(W_out, dtype=np.float32)
    x = oldx.reshape(-1, DIM).astype(np.float32)          # [8192, 1024]

    # node-major column permutation: our col 8n+t  <-  ref col 255t+n
    i = np.arange(WIDTH)
    perm = 255 * (i % PAR) + (i // PAR)

    w1t = np.zeros((DIM, NODES_PAD), np.float32)
    w1t[:, :WIDTH] = W_in[perm, :].T
    w1q = _round10(w1t[:, :CORR_COLS])
    dw = (w1t[:, :CORR_COLS].astype(np.float64) - w1q).astype(np.float32)

    def chunk_w(a, dt, ncols):
        return np.ascontiguousarray(
            np.asarray(a, np.float32)[:, :ncols]
            .reshape(K_CH, 128, ncols).transpose(1, 0, 2)).astype(dt)

    w1r = chunk_w(w1q * np.float32(2 ** 9), np.float32, CORR_COLS)
    w8 = chunk_w(np.asarray((w1t[:, :CORR_COLS] * 64).astype(e4),
                            np.float32), np.float32, CORR_COLS)
    dw8 = chunk_w(np.asarray((dw * SC).astype(e4), np.float32),
                  np.float32, CORR_COLS)
    w8p = np.ascontiguousarray(
        np.stack([w8, dw8], axis=2)).astype(e4)           # [128,K,2,1024]
    # leaf weights bf16, pre-scaled 2^9 (exact) to match the global 2^17
    w1h = chunk_w(
        np.asarray((w1t[:, CORR_COLS:NODES_PAD] * np.float32(2 ** 9))
                   .astype(ml_dtypes.bfloat16), np.float32),
        ml_dtypes.bfloat16, 1024)
    b1sv = np.zeros(NODES_PAD, np.float32)
    b1sv[:WIDTH] = b_in[perm] * np.float32(SC)

    w2t = np.zeros((NODES_PAD, DIM), np.float32)
    w2t[:WIDTH] = W_out.T[perm, :]
    w2 = np.ascontiguousarray(
        w2t.astype(np.float16).reshape(C_CH, 128, DIM).transpose(1, 0, 2))

    in_maps = []
    for c in range(N_CORES):
        xc = x[c * TOK_PER_CORE:(c + 1) * TOK_PER_CORE]   # [1024, 1024]
        xT = np.ascontiguousarray(xc.T)                   # [dim, tok]
        xq = _round10(xT)
        dx = (xT.astype(np.float64) - xq).astype(np.float32)

        def lay(a):
            return np.ascontiguousarray(
                np.asarray(a, np.float32)
                .reshape(K_CH, 128, NTILES, TT).transpose(1, 2, 0, 3))

        xsv = lay(xq * np.float32(2 ** 8))
        dx8 = lay(np.asarray((dx * 2048).astype(e4), np.float32))
        x8 = lay(np.asarray(xT.astype(e4), np.float32))
        x8pv = np.ascontiguousarray(
            np.stack([dx8, x8], axis=3)).astype(e4)       # [128,NT,K,2,TT]
        xhv = lay(np.asarray((xT * np.float32(2 ** 8))
                             .astype(ml_dtypes.bfloat16), np.float32)
                  ).astype(ml_dtypes.bfloat16)
        in_maps.append({
            "xs": xsv, "x8p": x8pv, "xh": xhv,
            "w1r": w1r, "w8p": w8p, "w1h": w1h,
            "b1s": b1sv, "w2": w2,
        })

    nc = _get_program()
    res = run_bass_kernel_spmd(nc, in_maps, core_ids=list(range(N_CORES)))
    out = np.concatenate([res.results[c]["y"] for c in range(N_CORES)],
                         axis=0)
    return out.reshape(oldx.shape).astype(np.float32)


# revision 34
# speedup vs baseline: 1.1664x; 1.0039x over previous
"""Trainium2 Bass kernel for the FFF (fast feedforward / MoE-routing) module.

Math (per token x of dim 1024, PAR=8 trees of 255 nodes):
  logits = x @ W_in.T + b_in                      # [B, 2040]
  dec    = logits > 0
  acts   = silu(logits)
  dmap   = indicator of the 8 visited nodes per tree
  out    = (acts * dmap) @ W_out.T                # [B, 1024]

Strategy (8 NeuronCores, data-parallel over the 8192 tokens, 1024 each):
  - Decision region (cols 0..1024) in fp32r (TF32-like: PE rounds both
    operands to 10 explicit mantissa bits at ~bf16 speed).  Inputs are
    pre-rounded to 10 bits on the host so the fp32r pass is EXACT; the
    known residual dx@w + x@dw is computed via fp8e4 DoubleRow matmuls
    (2 planes/instr, 2x rate at M=128) into a separate PSUM and merged at
    the bias-add.  Main pass pre-scaled by 2^17 (x*2^8, w*2^9) to match
    the fp8 plane scales; silu uses the activation unit's input scale.
    Net logit error ~5e-6 rms -> ~0 decision flips vs fp32 reference.
  - Leaf region (cols 1024..2048) single-pass bf16 (acts only need ~1e-3).
  - dmap built level-by-level with strided vector ops in node-major
    column layout (col = 8*node + tree).
  - masked acts in fp16, transposed on the PE, GEMM2 in fp16.
  - DMAs emitted in need-order; tiles 0-1 are hand-interleaved with the
    weight-arrival schedule so the PE has work while weights stream.
"""

import numpy as np
import ml_dtypes

DIM = 1024
PAR = 8
DEPTH = 7
N_NODES = 255
WIDTH = PAR * N_NODES          # 2040
NODES_PAD = 2048
N_CORES = 8
TOK_PER_CORE = 1024
TT = 128
NTILES = TOK_PER_CORE // TT    # 8
K_CH = DIM // 128              # 8
C_CH = NODES_PAD // 128        # 16
DEC_COLS = 8 * 127             # 1016
CORR_COLS = 1024               # fp8-corrected region (blocks 0,1)
SC = float(2 ** 17)

_PROGRAM = None


def _build_program():
    import concourse.bacc as bacc
    import concourse.tile as tile
    from concourse import mybir
    from concourse.masks import make_identity
    import concourse.bass as bass

    f32 = mybir.dt.float32
    f32r = mybir.dt.float32r
    bf16 = mybir.dt.bfloat16
    f16 = mybir.dt.float16
    fp8e4 = mybir.dt.float8e4
    Alu = mybir.AluOpType
    Act = mybir.ActivationFunctionType
    DRM = mybir.MatmulPerfMode.DoubleRow

    nc = bacc.Bacc("TRN2", target_bir_lowering=False, debug=False,
                   num_devices=N_CORES)

    xs = nc.dram_tensor("xs", [128, NTILES, K_CH, TT], f32r,
                        kind="ExternalInput")
    x8p = nc.dram_tensor("x8p", [128, NTILES, K_CH, 2, TT], fp8e4,
                         kind="ExternalInput")
    xh = nc.dram_tensor("xh", [128, NTILES, K_CH, TT], bf16,
                        kind="ExternalInput")
    w1r = nc.dram_tensor("w1r", [128, K_CH, CORR_COLS], f32r,
                         kind="ExternalInput")
    w8p = nc.dram_tensor("w8p", [128, K_CH, 2, CORR_COLS], fp8e4,
                         kind="ExternalInput")
    w1h = nc.dram_tensor("w1h", [128, K_CH, 1024], bf16,
                         kind="ExternalInput")
    b1s = nc.dram_tensor("b1s", [NODES_PAD], f32, kind="ExternalInput")
    w2 = nc.dram_tensor("w2", [128, C_CH, DIM], f16, kind="ExternalInput")
    y = nc.dram_tensor("y", [TOK_PER_CORE, DIM], f32, kind="ExternalOutput")

    with tile.TileContext(nc) as tc:
        with (
            tc.tile_pool(name="wts", bufs=1) as wts,
            tc.tile_pool(name="xts", bufs=4) as xts,
            tc.tile_pool(name="lgs", bufs=2) as lgs_pool,
            tc.tile_pool(name="d1p", bufs=4) as d1_pool,
            tc.tile_pool(name="vvp", bufs=2) as vv_pool,
            tc.tile_pool(name="acp", bufs=4) as ac_pool,
            tc.tile_pool(name="mkp", bufs=2) as mk_pool,
            tc.tile_pool(name="out", bufs=2) as out_pool,
            tc.tile_pool(name="pl", bufs=3, space="PSUM") as pl_pool,
            tc.tile_pool(name="pc", bufs=1, space="PSUM") as pc_pool,
            tc.tile_pool(name="pt", bufs=2, space="PSUM") as pt_pool,
            tc.tile_pool(name="py", bufs=2, space="PSUM") as py_pool,
        ):
            # Weight tiles are split per DMA batch: the Tile framework
            # tracks dependencies at tile granularity, so a consumer waits
            # for ALL writes to its tile — separate tiles let the first
            # matmuls start as soon as their own bytes land.
            w1r_b0a = wts.tile([128, 2, 512], f32r)       # b0, k 0-1
            w1r_b0b = wts.tile([128, 6, 512], f32r)       # b0, k 2-7
            w1r_b1 = wts.tile([128, K_CH, 512], f32r)
            w8p_a = wts.tile([128, 4, 2, CORR_COLS], fp8e4)   # k 0-3
            w8p_b = wts.tile([128, 4, 2, CORR_COLS], fp8e4)   # k 4-7
            w1h_b2 = wts.tile([128, K_CH, 512], bf16)
            w1h_b3 = wts.tile([128, K_CH, 512], bf16)
            w2_sb = wts.tile([128, C_CH, DIM], f16)
            b1s_sb = wts.tile([128, NODES_PAD], f32)
            ident = wts.tile([128, 128], f16)

            def w1r_at(k, b):
                if b == 1:
                    return w1r_b1[:, k, :]
                return (w1r_b0a[:, k, :] if k < 2 else w1r_b0b[:, k - 2, :])

            def w8p_at(k, cols):
                t = w8p_a if k < 4 else w8p_b
                return t[:, k % 4, :, cols]

            def w1h_at(k, b):
                return (w1h_b2 if b == 2 else w1h_b3)[:, k, :]

            xt_tiles = {}

            def fetch_xs(j, eng):
                xsj = xts.tile([128, K_CH, TT], f32r, tag="xs")
                eng.dma_start(out=xsj, in_=xs[:, j])
                return xsj

            def fetch_x8(j, eng):
                x8j = xts.tile([128, K_CH, 2, TT], fp8e4, tag="x8")
                eng.dma_start(out=x8j, in_=x8p[:, j])
                return x8j

            def fetch_xh(j, eng):
                xhj = xts.tile([128, K_CH, TT], bf16, tag="xh")
                eng.dma_start(out=xhj, in_=xh[:, j])
                return xhj

            def prefetch_xt(j, eng=None):
                e = eng or nc.gpsimd
                xt_tiles[j] = (fetch_xs(j, e), fetch_x8(j, e), fetch_xh(j, e))

            # bias broadcast + identity off the Sync weight path, early
            b1_bcast = bass.AP(tensor=b1s, offset=0,
                               ap=[[0, 128], [1, NODES_PAD]])
            nc.gpsimd.dma_start(out=b1s_sb, in_=b1_bcast)
            make_identity(nc, ident)

            # per-tile stage-A state
            st = {}

            def a_init(j):
                if j not in xt_tiles:
                    prefetch_xt(j)
                d1 = d1_pool.tile([TT, DEC_COLS], f16, tag="d1")
                vv = vv_pool.tile([TT, WIDTH], f16, tag="vv")
                ac = ac_pool.tile([TT, NODES_PAD], f16, tag="ac")
                st[j] = {"x": xt_tiles.pop(j), "d1": d1, "vv": vv, "ac": ac,
                         "pl": {}, "pc": {}}

            def a_main(j, b):
                """fp32r main pass for block b (512 cols), group closed."""
                s = st[j]
                xsat = s.get("xsat") or (lambda k: s["x"][0][:, k, :])
                pl = pl_pool.tile([TT, 512], f32)
                for k in range(K_CH):
                    nc.tensor.matmul(pl, lhsT=xsat(k),
                                     rhs=w1r_at(k, b),
                                     start=(k == 0), stop=(k == K_CH - 1))
                s["pl"][b] = pl

            def a_corr(j, b):
                """fp8 DR correction for block b into its own PSUM."""
                s = st[j]
                x8j = s["x"][1]
                lo = b * 512
                pc = pc_pool.tile([TT, 512], f32)
                for q in range(2):
                    qs = slice(q * 256, (q + 1) * 256)
                    wq = slice(lo + q * 256, lo + (q + 1) * 256)
                    for k in range(K_CH):
                        nc.tensor.matmul(
                            pc[:, qs], lhsT=x8j[:, k, :, :],
                            rhs=w8p_at(k, wq),
                            start=(k == 0), stop=(k == K_CH - 1),
                            perf_mode=DRM, skip_group_check=True)
                s["pc"][b] = pc

            def a_leaf(j, b):
                """bf16 leaf pass for block b (2 or 3), group closed."""
                s = st[j]
                xhj = s["x"][2]
                pl = pl_pool.tile([TT, 512], f32)
                for k in range(K_CH):
                    nc.tensor.matmul(pl, lhsT=xhj[:, k, :],
                                     rhs=w1h_at(k, b),
                                     start=(k == 0), stop=(k == K_CH - 1))
                s["pl"][b] = pl

            def a_epi(j, b):
                """bias (+corr) add, decisions, silu for block b."""
                s = st[j]
                lo = b * 512
                pl = s["pl"].pop(b)
                lg = lgs_pool.tile([TT, 512], f32, tag="lg")
                if b in s["pc"]:
                    # vector ops may read only one PSUM operand each:
                    # lg = (pc + bias) then lg += pl
                    pc = s["pc"].pop(b)
                    nc.vector.tensor_tensor(lg, pc, b1s_sb[:, lo:lo + 512],
                                            Alu.add)
                    nc.vector.tensor_tensor(lg, lg, pl, Alu.add)
                else:
                    nc.vector.tensor_tensor(lg, pl, b1s_sb[:, lo:lo + 512],
                                            Alu.add)
                if b == 0:
                    nc.vector.tensor_scalar(s["d1"][:, 0:512], lg, 0.0, None,
                                            Alu.is_gt)
                elif b == 1:
                    nc.vector.tensor_scalar(s["d1"][:, 512:DEC_COLS],
                                            lg[:, 0:DEC_COLS - 512],
                                            0.0, None, Alu.is_gt)
                # leaf blocks: silu over the whole 512 (incl. pad cols; mk
                # zeroes the 2040:2048 tail later)
                nc.scalar.activation(s["ac"][:, lo:lo + 512], lg, Act.Silu,
                                     scale=1.0 / SC)

            def a_mask(j):
                """tree traversal mask + masked acts (fp16)."""
                s = st[j]
                d1, vv, ac = s["d1"], s["vv"], s["ac"]
                mk = mk_pool.tile([TT, NODES_PAD], f16, tag="mk")
                nc.vector.memset(vv[:, 0:8], 1.0)
                for d in range(DEPTH):
                    ld = 8 * (1 << d)
                    c0 = 8 * ((1 << d) - 1)
                    c1 = 8 * ((1 << (d + 1)) - 1)
                    vpar = vv[:, c0:c0 + ld].rearrange("p (i t) -> p i t", t=8)
                    dpar = d1[:, c0:c0 + ld].rearrange("p (i t) -> p i t", t=8)
                    kids = vv[:, c1:c1 + 2 * ld].rearrange(
                        "p (i two t) -> p i two t", two=2, t=8)
                    nc.vector.tensor_tensor(kids[:, :, 1, :], vpar, dpar,
                                            Alu.mult)
                    nc.vector.tensor_tensor(kids[:, :, 0, :], vpar,
                                            kids[:, :, 1, :], Alu.subtract)
                nc.vector.memset(mk[:, WIDTH:NODES_PAD], 0.0)
                # first 128 cols split out so stage_b's first transpose
                # can start before the rest of the masking finishes
                nc.vector.tensor_tensor(mk[:, 0:128], ac[:, 0:128],
                                        vv[:, 0:128], Alu.mult)
                nc.vector.tensor_tensor(mk[:, 128:1024], ac[:, 128:1024],
                                        vv[:, 128:1024], Alu.mult)
                nc.vector.tensor_tensor(mk[:, 1024:WIDTH], ac[:, 1024:WIDTH],
                                        vv[:, 1024:WIDTH], Alu.mult)
                s["mk"] = mk

            def a_fused(j, b):
                """steady state: fp32r + DR corrections in ONE PSUM group
                (weights resident, no need for the ramp's split-PSUM)."""
                s = st[j]
                xsat = s.get("xsat") or (lambda k: s["x"][0][:, k, :])
                x8j = s["x"][1]
                lo = b * 512
                pl = pl_pool.tile([TT, 512], f32)
                for k in range(K_CH):
                    nc.tensor.matmul(pl, lhsT=xsat(k), rhs=w1r_at(k, b),
                                     start=(k == 0), stop=False)
                for q in range(2):
                    qs = slice(q * 256, (q + 1) * 256)
                    wq = slice(lo + q * 256, lo + (q + 1) * 256)
                    for k in range(K_CH):
                        nc.tensor.matmul(
                            pl[:, qs], lhsT=x8j[:, k, :, :],
                            rhs=w8p_at(k, wq),
                            start=False, stop=(q == 1 and k == K_CH - 1),
                            perf_mode=DRM, skip_group_check=True)
                s["pl"][b] = pl

            def a_full(j):
                a_init(j)
                a_fused(j, 0)
                a_epi(j, 0)
                a_fused(j, 1)
                a_epi(j, 1)
                a_leaf(j, 2)
                a_epi(j, 2)
                a_leaf(j, 3)
                a_epi(j, 3)
                a_mask(j)

            def stage_b(j):
                s = st.pop(j)
                mk = s["mk"]
                at = mk_pool.tile([128, C_CH, TT], f16, tag="at")
                c = 0
                for gsz in (1, 3, 4, 4, 4):
                    pt = pt_pool.tile([128, 512], f16)
                    for i in range(gsz):
                        nc.tensor.transpose(
                            pt[:, i * 128:(i + 1) * 128],
                            mk[:, (c + i) * 128:(c + i + 1) * 128], ident)
                    nc.scalar.copy(
                        at[:, c:c + gsz, :],
                        pt[:, :gsz * 128].rearrange("p (c t) -> p c t", t=TT))
                    c += gsz
                ys = out_pool.tile([TT, DIM], f32, tag="ys")
                hw = DIM // 2
                for h in range(2):
                    hs = slice(h * hw, (h + 1) * hw)
                    py = py_pool.tile([TT, hw], f32)
                    for c in range(C_CH):
                        nc.tensor.matmul(
                            py, lhsT=at[:, c, :], rhs=w2_sb[:, c, hs],
                            start=(c == 0), stop=(c == C_CH - 1))
                    nc.vector.tensor_copy(ys[:, hs], py)
                    nc.sync.dma_start(out=y[j * TT:(j + 1) * TT, hs],
                                      in_=ys[:, hs])

            # ---- ramp: DMAs emitted at point-of-need, 4 tiles of b0/b1
            # decision work interleaved with the weight stream, leaves and
            # GEMM2 deferred until their weights can have arrived ----
            nc.sync.dma_start(out=w1r_b0a, in_=w1r[:, 0:2, 0:512])
            xs0 = fetch_xs(0, nc.sync)
            nc.sync.dma_start(out=w1r_b0b, in_=w1r[:, 2:8, 0:512])
            xt_tiles[0] = (xs0, None, None)
            a_init(0)
            a_main(0, 0)
            x80 = fetch_x8(0, nc.sync)
            st[0]["x"] = (xs0, x80, None)
            nc.sync.dma_start(out=w8p_a, in_=w8p[:, 0:4])
            nc.sync.dma_start(out=w8p_b, in_=w8p[:, 4:8])
            a_corr(0, 0)
            a_epi(0, 0)
            xs1 = fetch_xs(1, nc.sync)
            x81 = fetch_x8(1, nc.sync)
            xt_tiles[1] = (xs1, x81, None)
            a_init(1)
            a_main(1, 0)
            nc.sync.dma_start(out=w1r_b1, in_=w1r[:, :, 512:1024])
            a_main(0, 1)
            a_corr(0, 1)
            a_epi(0, 1)
            a_main(1, 1)
            a_corr(1, 0)
            a_epi(1, 0)
            a_corr(1, 1)
            a_epi(1, 1)
            xs2 = fetch_xs(2, nc.sync)
            x82 = fetch_x8(2, nc.sync)
            xt_tiles[2] = (xs2, x82, None)
            a_init(2)
            a_main(2, 0)
            a_main(2, 1)
            nc.sync.dma_start(out=w1h_b2, in_=w1h[:, :, 0:512])
            a_corr(2, 0)
            a_epi(2, 0)
            a_corr(2, 1)
            a_epi(2, 1)
            xs3 = fetch_xs(3, nc.sync)
            x83 = fetch_x8(3, nc.sync)
            xt_tiles[3] = (xs3, x83, None)
            a_init(3)
            a_main(3, 0)
            a_main(3, 1)
            nc.sync.dma_start(out=w1h_b3, in_=w1h[:, :, 512:1024])
            st[0]["x"] = (st[0]["x"][0], st[0]["x"][1], fetch_xh(0, nc.sync))
            st[1]["x"] = (st[1]["x"][0], st[1]["x"][1], fetch_xh(1, nc.sync))
            nc.sync.dma_start(out=w2_sb[:, 0:4, :], in_=w2[:, 0:4, :])
            nc.sync.dma_start(out=w2_sb[:, 4:8, :], in_=w2[:, 4:8, :])
            nc.sync.dma_start(out=w2_sb[:, 8:12, :], in_=w2[:, 8:12, :])
            nc.sync.dma_start(out=w2_sb[:, 12:16, :], in_=w2[:, 12:16, :])
            a_corr(3, 0)
            a_epi(3, 0)
            a_corr(3, 1)
            a_epi(3, 1)
            a_leaf(0, 2)
            a_epi(0, 2)
            st[2]["x"] = (st[2]["x"][0], st[2]["x"][1], fetch_xh(2, nc.sync))
            st[3]["x"] = (st[3]["x"][0], st[3]["x"][1], fetch_xh(3, nc.sync))
            a_leaf(0, 3)
            a_epi(0, 3)
            a_mask(0)
            a_leaf(1, 2)
            a_epi(1, 2)
            a_leaf(1, 3)
            a_epi(1, 3)
            a_mask(1)
            stage_b(0)
            a_leaf(2, 2)
            a_epi(2, 2)
            a_leaf(2, 3)
            a_epi(2, 3)
            a_mask(2)
            prefetch_xt(4)
            stage_b(1)
            a_leaf(3, 2)
            a_epi(3, 2)
            a_leaf(3, 3)
            a_epi(3, 3)
            a_mask(3)
            stage_b(2)
            # ---- steady state ----
            for j in range(4, NTILES):
                if j + 1 < NTILES:
                    prefetch_xt(j + 1)
                a_full(j)
                stage_b(j - 1)
            stage_b(NTILES - 1)

    nc.finalize()
    return nc


def _get_program():
    global _PROGRAM
    if _PROGRAM is None:
        _PROGRAM = _build_program()
    return _PROGRAM


def _round10(a):
    """Round fp32 to 10 explicit mantissa bits (RNE) — fp32r's internal
    operand precision, so the PE's own rounding becomes the identity."""
    bits = np.asarray(a, np.float32).view(np.uint32).copy()
    lsb = (bits >> np.uint32(13)) & np.uint32(1)
    bits = bits + np.uint32((1 << 12) - 1) + lsb
    bits &= np.uint32(~((1 << 13) - 1) & 0xFFFFFFFF)
    return bits.view(np.float32)


def kernel(oldx, W_in, b_in, W_out):
    from concourse.bass_utils import run_bass_kernel_spmd

    e4 = ml_dtypes.float8_e4m3
    oldx = np.asarray(oldx)
    W_in = np.asarray(W_in, dtype=np.float32)
    b_in = np.asarray(b_in, dtype=np.float32)
    W_out = np.asarray(W_out, dtype=np.float32)
    x = oldx.reshape(-1, DIM).astype(np.float32)          # [8192, 1024]

    # node-major column permutation: our col 8n+t  <-  ref col 255t+n
    i = np.arange(WIDTH)
    perm = 255 * (i % PAR) + (i // PAR)

    w1t = np.zeros((DIM, NODES_PAD), np.float32)
    w1t[:, :WIDTH] = W_in[perm, :].T
    w1q = _round10(w1t[:, :CORR_COLS])
    dw = (w1t[:, :CORR_COLS].astype(np.float64) - w1q).astype(np.float32)

    def chunk_w(a, dt, ncols):
        return np.ascontiguousarray(
            np.asarray(a, np.float32)[:, :ncols]
            .reshape(K_CH, 128, ncols).transpose(1, 0, 2)).astype(dt)

    w1r = chunk_w(w1q * np.float32(2 ** 9), np.float32, CORR_COLS)
    w8 = chunk_w(np.asarray((w1t[:, :CORR_COLS] * 64).astype(e4),
                            np.float32), np.float32, CORR_COLS)
    dw8 = chunk_w(np.asarray((dw * SC).astype(e4), np.float32),
                  np.float32, CORR_COLS)
    w8p = np.ascontiguousarray(
        np.stack([w8, dw8], axis=2)).astype(e4)           # [128,K,2,1024]
    # leaf weights bf16, pre-scaled 2^9 (exact) to match the global 2^17
    w1h = chunk_w(
        np.asarray((w1t[:, CORR_COLS:NODES_PAD] * np.float32(2 ** 9))
                   .astype(ml_dtypes.bfloat16), np.float32),
        ml_dtypes.bfloat16, 1024)
    b1sv = np.zeros(NODES_PAD, np.float32)
    b1sv[:WIDTH] = b_in[perm] * np.float32(SC)

    w2t = np.zeros((NODES_PAD, DIM), np.float32)
    w2t[:WIDTH] = W_out.T[perm, :]
    w2 = np.ascontiguousarray(
        w2t.astype(np.float16).reshape(C_CH, 128, DIM).transpose(1, 0, 2))

    in_maps = []
    for c in range(N_CORES):
        xc = x[c * TOK_PER_CORE:(c + 1) * TOK_PER_CORE]   # [1024, 1024]
        xT = np.ascontiguousarray(xc.T)                   # [dim, tok]
        xq = _round10(xT)
        dx = (xT.astype(np.float64) - xq).astype(np.float32)

        def lay(a):
            return np.ascontiguousarray(
                np.asarray(a, np.float32)
                .reshape(K_CH, 128, NTILES, TT).transpose(1, 2, 0, 3))

        xsv = lay(xq * np.float32(2 ** 8))
        dx8 = lay(np.asarray((dx * 2048).astype(e4), np.float32))
        x8 = lay(np.asarray(xT.astype(e4), np.float32))
        x8pv = np.ascontiguousarray(
            np.stack([dx8, x8], axis=3)).astype(e4)       # [128,NT,K,2,TT]
        xhv = lay(np.asarray((xT * np.float32(2 ** 8))
                             .astype(ml_dtypes.bfloat16), np.float32)
                  ).astype(ml_dtypes.bfloat16)
        in_maps.append({
            "xs": xsv, "x8p": x8pv, "xh": xhv,
            "w1r": w1r, "w8p": w8p, "w1h": w1h,
            "b1s": b1sv, "w2": w2,
        })

    nc = _get_program()
    res = run_bass_kernel_spmd(nc, in_maps, core_ids=list(range(N_CORES)))
    out = np.concatenate([res.results[c]["y"] for c in range(N_CORES)],
                         axis=0)
    return out.reshape(oldx.shape).astype(np.float32)
